# revision 3
# baseline (speedup 1.0000x reference)
"""DeepHit loss kernel for Trainium2 (8 NeuronCores, Bass/Tile).

Math
----
reference:
    p   = clip(preds, 1e-12, 1-1e-12)            [B, T]
    d_i = clip(durations_i - 1, 0, T-1)
    t_i = p[i, d_i]
    lik = -log(t_i) * ev_i                       (weights are all 1.0)
    rank_sum = sum_{i,j} relu(p[j, d_i] - t_i) * [d_j > d_i] * [ev_i = 1]
    count    = #{(i,j) : d_j > d_i, ev_i = 1}
    out = 0.5 * mean(lik) + 0.5 * rank_sum / count

Device reformulation (the only O(B^2) term is rank_sum):
    durations take T=64 distinct values, so the gather p[j, d_i] is a
    one-hot matmul.  Let
        Wm[c, j] = p[j, c] * [d_j > c]            (mask folded into columns)
        E [c, i] = [d_i == c]                     (one-hot; i = event rows)
        bias_i   = -t_i
    then
        relu((E^T @ Wm)[i, j] + bias_i)
          = relu(p[j,d_i] * [d_j > d_i] - t_i)
          = relu(p[j,d_i] - t_i) * [d_j > d_i]
    because t_i > 0 makes the masked case (-t_i) vanish under relu.
    rank_sum is the global sum of that matrix over event rows i.

    On device: i on PSUM partitions (so bias_i is a per-partition ACT bias),
    j on the free dim.  The matmul runs in fp16 hi/lo split (exact to
    ~2^-24 relative since E is exactly representable): K packs [Whi; Wlo]
    against [E; E] so one K=128 matmul does hi+lo in a single pass.
    The relu+bias+sum runs fused on ScalarE (activation accum_out) and
    VectorE (scalar_tensor_tensor accum_out), alternating tiles.

Sharding + work reduction:
    * only event rows appear on the i side (~B/2 of them);
    * rows are sorted by duration (host permutation; rank_sum and count
      are permutation invariant) and event rows are dealt round-robin to
      the 8 cores so every core sees the same duration profile;
    * per i-tile, j is restricted to the suffix of sorted rows with
      d_j > min(d_i of the tile); the per-element Wm mask keeps the
      boundary exact.
    Each core returns [128, n_slots] partial sums; the host adds them and
    combines with the O(B) NLL/count terms (host-side marshalling only).
"""

import sys

sys.path.insert(0, "/opt/trn_rl_repo")

import numpy as np

import concourse.bacc as bacc
import concourse.mybir as mybir
import concourse.tile as tile
from concourse.bass_utils import run_bass_kernel_spmd

B = 8192
T = 64
NCORES = 8
ITILE = 128          # i rows per PSUM tile (partition dim)
JSUP = 2048          # j columns per consume instruction (4 PSUM banks)
JMM = 512            # j columns per matmul (1 PSUM bank)

f16 = mybir.dt.float16
f32 = mybir.dt.float32
bf16 = mybir.dt.bfloat16

_cache = {}


def _build_program(n_itiles, jlims, repeat=1):
    """Build + compile the SPMD bass program.

    n_itiles: i-tiles per core (each 128 partitions).
    jlims[t]: j extent (multiple of JSUP) needed by i-tile t; the tile's
              matmuls cover the LAST jlims[t] columns of wstack.  The
              structure is identical on every core; cores differ in data.
    repeat:   emit the whole body N times (timing variants only).
    """
    nc = bacc.Bacc(
        "TRN2", target_bir_lowering=False, debug=False, num_devices=NCORES
    )

    blk = n_itiles * ITILE
    jmax = max(jlims)
    n_slots = sum(j // JSUP for j in jlims)
    half = (n_slots + 1) // 2

    wstack_d = nc.dram_tensor("wstack", [128, jmax], f16, kind="ExternalInput")
    estack_d = nc.dram_tensor("estack", [128, blk], f16, kind="ExternalInput")
    negt_d = nc.dram_tensor("negt", [128, n_itiles], f32, kind="ExternalInput")
    part_d = nc.dram_tensor("partials", [128, 2 * half], f32, kind="ExternalOutput")

    with tile.TileContext(nc) as tc:
        with (
            tc.tile_pool(name="const", bufs=1) as zpool,
            tc.tile_pool(name="inp", bufs=min(2, max(1, repeat))) as cpool,
            tc.tile_pool(name="psum", bufs=2, space="PSUM") as ppool,
            tc.tile_pool(name="scr", bufs=3) as spool,
        ):
            zeros = zpool.tile([128, JSUP], bf16)
            nc.vector.memset(zeros[:], 0.0)

            for _rep in range(repeat):
                wstack = cpool.tile([128, jmax], f16, tag="wstack")
                # chunked load so early matmuls overlap the tail of the
                # DMA; high j (large durations) is needed by every i-tile,
                # so load back-to-front
                for j0 in reversed(range(0, jmax, JSUP)):
                    nc.sync.dma_start(
                        wstack[:, j0 : j0 + JSUP], wstack_d[:, j0 : j0 + JSUP]
                    )
                estack = cpool.tile([128, blk], f16, tag="estack")
                nc.sync.dma_start(estack[:], estack_d[:])
                negt = cpool.tile([128, n_itiles], f32, tag="negt")
                nc.sync.dma_start(negt[:], negt_d[:])
                # separate accumulator tiles per engine so ACT/DVE never
                # share a written tile (keeps the two streams independent)
                acc_act = cpool.tile([128, half], f32, tag="acc_act")
                acc_dve = cpool.tile([128, half], f32, tag="acc_dve")

                slot = 0
                for it in range(n_itiles):
                    lhsT = estack[:, it * ITILE : (it + 1) * ITILE]
                    bias = negt[:, it : it + 1]
                    for js in range((jmax - jlims[it]) // JSUP, jmax // JSUP):
                        ps = ppool.tile([128, JSUP], f32, tag="ps")
                        for q in range(JSUP // JMM):
                            j0 = js * JSUP + q * JMM
                            nc.tensor.matmul(
                                ps[:, q * JMM : (q + 1) * JMM],
                                lhsT,
                                wstack[:, j0 : j0 + JMM],
                                start=True,
                                stop=True,
                            )
                        scr = spool.tile([128, JSUP], bf16, tag="scr")
                        if slot % 2 == 0:
                            nc.scalar.activation(
                                scr[:],
                                ps[:],
                                mybir.ActivationFunctionType.Relu,
                                bias=bias,
                                scale=1.0,
                                accum_out=acc_act[:, slot // 2 : slot // 2 + 1],
                            )
                        else:
                            nc.vector.scalar_tensor_tensor(
                                scr[:],
                                ps[:],
                                bias,
                                zeros[:],
                                op0=mybir.AluOpType.add,
                                op1=mybir.AluOpType.max,
                                accum_out=acc_dve[:, slot // 2 : slot // 2 + 1],
                            )
                        slot += 1
                if n_slots % 2 == 1:
                    # odd slot count leaves the last dve column unwritten
                    nc.vector.memset(acc_dve[:, half - 1 : half], 0.0)
                nc.sync.dma_start(part_d[:, :half], acc_act[:])
                nc.sync.dma_start(part_d[:, half:], acc_dve[:])

    nc.compile()
    return nc


def _prep(preds, durations, events):
    """Host-side marshalling: sort by duration, build the one-hot/mask/bias
    operands, fp16 hi/lo split, and the O(B) scalar terms."""
    p = np.clip(np.asarray(preds, dtype=np.float32), 1e-12, 1.0 - 1e-12)
    dur = np.asarray(durations)
    ev = np.asarray(events, dtype=np.float32)
    Bn, Tn = p.shape

    d = np.clip(dur.astype(np.int64) - 1, 0, Tn - 1)
    t = p[np.arange(Bn), d]

    # O(B) host terms
    lik_sum = float(np.sum(-np.log(t.astype(np.float64)) * ev.astype(np.float64)))
    hist = np.bincount(d, minlength=Tn)
    gtc = np.zeros(Tn, np.int64)
    gtc[:-1] = hist[::-1].cumsum()[::-1][1:]  # gtc[c] = #{j : d_j > c}
    count = int((ev.astype(np.int64) * gtc[d]).sum())

    # sort rows by duration (stable); the j side keeps all rows
    order = np.argsort(d, kind="stable")
    d_s = d[order]
    p_s = p[order]

    cbins = np.arange(Tn)
    Wm = np.where(d_s[None, :] > cbins[:, None], p_s.T, np.float32(0.0)).astype(
        np.float32
    )
    Whi = Wm.astype(np.float16)
    Wlo = (Wm - Whi.astype(np.float32)).astype(np.float16)
    wstack_full = np.concatenate([Whi, Wlo], axis=0)  # [128, B]

    # i side: event rows only, sorted order, dealt round-robin to cores
    ev_s = ev[order]
    t_s = t[order]
    ev_pos = np.nonzero(ev_s == 1)[0]
    nev = len(ev_pos)

    ev_per_core = (nev + NCORES - 1) // NCORES
    n_itiles = max(1, (ev_per_core + ITILE - 1) // ITILE)
    blk = n_itiles * ITILE

    # per-core sorted event durations / thresholds, padded with d=Tn, t=0
    d_i = np.full((NCORES, blk), Tn, np.int64)
    t_i = np.zeros((NCORES, blk), np.float32)
    for c in range(NCORES):
        pos = ev_pos[c::NCORES]
        d_i[c, : len(pos)] = d_s[pos]
        t_i[c, : len(pos)] = t_s[pos]

    # first_gt[c] = first sorted j with d_j > c
    first_gt = np.searchsorted(d_s, np.arange(Tn), side="right")

    # per i-tile j extent (max over cores, rounded up to JSUP)
    jlims = []
    for tt in range(n_itiles):
        need = JSUP
        for c in range(NCORES):
            dmin = int(d_i[c, tt * ITILE : (tt + 1) * ITILE].min())
            if dmin < Tn:
                n = Bn - int(first_gt[dmin])
                need = max(need, ((n + JSUP - 1) // JSUP) * JSUP)
        jlims.append(min(need, Bn))
    jmax = max(jlims)

    # device wstack holds the last jmax sorted rows
    wstack = np.ascontiguousarray(wstack_full[:, Bn - jmax :])

    in_maps = []
    for c in range(NCORES):
        E = (d_i[c][None, :] == cbins[:, None]).astype(np.float16)  # [T, blk]
        estack = np.ascontiguousarray(np.concatenate([E, E], axis=0))
        negt = np.ascontiguousarray(
            (-t_i[c]).reshape(n_itiles, ITILE).T
        )  # [128, n_itiles]
        in_maps.append({"wstack": wstack, "estack": estack, "negt": negt})
    return in_maps, n_itiles, jlims, lik_sum, count, Bn


def kernel(preds, durations, events):
    in_maps, n_itiles, jlims, lik_sum, count, Bn = _prep(preds, durations, events)

    key = (n_itiles, tuple(jlims))
    if key not in _cache:
        _cache[key] = _build_program(n_itiles, jlims)
    nc = _cache[key]

    res = run_bass_kernel_spmd(nc, in_maps, core_ids=list(range(NCORES)))
    rank_sum = 0.0
    for r in res.results:
        rank_sum += float(r["partials"].astype(np.float64).sum())

    rank = rank_sum / count if count > 0 else 0.0
    total = 0.5 * (lik_sum / Bn) + 0.5 * rank
    return np.array(total, dtype=np.float32)


# revision 9
# speedup vs baseline: 15.9315x; 15.9315x over previous
"""DeepHit loss kernel for Trainium2 (8 NeuronCores, Bass/Tile).

Math
----
reference:
    p   = clip(preds, 1e-12, 1-1e-12)            [B, T]
    d_i = clip(durations_i - 1, 0, T-1)
    t_i = p[i, d_i]
    lik = -log(t_i) * ev_i                       (weights are all 1.0)
    rank_sum = sum_{i,j} relu(p[j, d_i] - t_i) * [d_j > d_i] * [ev_i = 1]
    count    = #{(i,j) : d_j > d_i, ev_i = 1}
    out = 0.5 * mean(lik) + 0.5 * rank_sum / count

Device reformulation (the only O(B^2) term is rank_sum):
    durations take T=64 distinct values, so the gather p[j, d_i] is a
    one-hot matmul.  Let
        Wm[c, j] = p[j, c] * [d_j > c]            (mask folded into columns)
        E [c, i] = [d_i == c]                     (one-hot; i = event rows)
        bias_i   = -t_i
    then
        relu((E^T @ Wm)[i, j] + bias_i)
          = relu(p[j,d_i] * [d_j > d_i] - t_i)
          = relu(p[j,d_i] - t_i) * [d_j > d_i]
    because t_i > 0 makes the masked case (-t_i) vanish under relu.
    rank_sum is the global sum of that matrix over event rows i.

    On device: i on PSUM partitions (so bias_i is a per-partition ACT bias),
    j on the free dim.  The matmul runs in fp16 hi/lo split (exact to
    ~2^-24 relative since E is exactly representable): K packs [Whi; Wlo]
    against [E; E] so one K=128 matmul does hi+lo in a single pass.
    The relu+bias+sum runs fused on ScalarE (activation accum_out) and
    VectorE (scalar_tensor_tensor accum_out), alternating tiles.

Sharding + work reduction:
    * only event rows appear on the i side (~B/2 of them);
    * rows are sorted by duration (host permutation; rank_sum and count
      are permutation invariant) and event rows are dealt round-robin to
      the 8 cores so every core sees the same duration profile;
    * per i-tile, j is restricted to the suffix of sorted rows with
      d_j > min(d_i of the tile); the per-element Wm mask keeps the
      boundary exact.
    Each core returns [128, n_slots] partial sums; the host adds them and
    combines with the O(B) NLL/count terms (host-side marshalling only).
"""

import sys

sys.path.insert(0, "/opt/trn_rl_repo")

import numpy as np

import concourse.bacc as bacc
import concourse.mybir as mybir
import concourse.tile as tile
from concourse.bass_utils import run_bass_kernel_spmd

B = 8192
T = 64
NCORES = 8
ITILE = 128          # i rows per PSUM tile (partition dim)
JSUP = 2048          # j columns per consume instruction (4 PSUM banks)
JMM = 512            # j columns per matmul (1 PSUM bank)

f16 = mybir.dt.float16
f32 = mybir.dt.float32
bf16 = mybir.dt.bfloat16

_cache = {}


def _build_program(n_itiles, jlims, repeat=1):
    """Build + compile the SPMD bass program.

    n_itiles: i-tiles per core (each 128 partitions).
    jlims[t]: j extent (multiple of JSUP) needed by i-tile t; the tile's
              matmuls cover the LAST jlims[t] columns of wstack.  The
              structure is identical on every core; cores differ in data.
    repeat:   emit the whole body N times (timing variants only).
    """
    nc = bacc.Bacc(
        "TRN2", target_bir_lowering=False, debug=False, num_devices=NCORES
    )

    blk = n_itiles * ITILE
    jmax = max(jlims)
    # per tile: list of (super_base, offset_within_super); the consume
    # covers columns [super_base + offset, super_base + JSUP)
    tile_supers = []
    n_slots = 0
    for jl in jlims:
        start = jmax - jl  # multiple of JMM
        sbase0 = (start // JSUP) * JSUP
        supers = []
        for sbase in range(sbase0, jmax, JSUP):
            off = max(0, start - sbase)
            supers.append((sbase, off))
        tile_supers.append(supers)
        n_slots += len(supers)
    half = (n_slots + 1) // 2

    wstack_d = nc.dram_tensor("wstack", [128, jmax], f16, kind="ExternalInput")
    estack_d = nc.dram_tensor("estack", [128, blk], f16, kind="ExternalInput")
    negt_d = nc.dram_tensor("negt", [128, n_itiles], f32, kind="ExternalInput")
    part_d = nc.dram_tensor("partials", [128, 2 * half], f32, kind="ExternalOutput")

    with tile.TileContext(nc) as tc:
        with (
            tc.tile_pool(name="const", bufs=1) as zpool,
            tc.tile_pool(name="inp", bufs=min(2, max(1, repeat))) as cpool,
            tc.tile_pool(name="psum", bufs=2, space="PSUM") as ppool,
            tc.tile_pool(name="scr_a", bufs=2) as spool_a,
            tc.tile_pool(name="scr_d", bufs=2) as spool_d,
        ):
            zeros = zpool.tile([128, JSUP], bf16)
            nc.vector.memset(zeros[:], 0.0)
            # dummy activation with no data deps: pulls the ~2.7us Relu
            # table load to kernel start, hidden under the input DMA
            warm = zpool.tile([128, 1], f32)
            nc.scalar.activation(
                warm[:], zeros[:, :1], mybir.ActivationFunctionType.Relu
            )

            for _rep in range(repeat):
                wstack = cpool.tile([128, jmax], f16, tag="wstack")
                # chunked load so early matmuls overlap the tail of the
                # DMA; high j (large durations) is needed by every i-tile,
                # so load back-to-front
                for j0 in reversed(range(0, jmax, JSUP)):
                    nc.sync.dma_start(
                        wstack[:, j0 : j0 + JSUP], wstack_d[:, j0 : j0 + JSUP]
                    )
                estack = cpool.tile([128, blk], f16, tag="estack")
                nc.sync.dma_start(estack[:], estack_d[:])
                negt = cpool.tile([128, n_itiles], f32, tag="negt")
                nc.sync.dma_start(negt[:], negt_d[:])
                # separate accumulator tiles per engine so ACT/DVE never
                # share a written tile (keeps the two streams independent)
                acc_act = cpool.tile([128, half], f32, tag="acc_act")
                acc_dve = cpool.tile([128, half], f32, tag="acc_dve")

                slot = 0
                for it in range(n_itiles):
                    lhsT = estack[:, it * ITILE : (it + 1) * ITILE]
                    bias = negt[:, it : it + 1]
                    # high j first: those wstack chunks are DMA'd first, and
                    # every i-tile needs the high end
                    for sbase, off in reversed(tile_supers[it]):
                        width = JSUP - off
                        ps = ppool.tile([128, JSUP], f32, tag="ps")
                        for q in range(off // JMM, JSUP // JMM):
                            j0 = sbase + q * JMM
                            nc.tensor.matmul(
                                ps[:, q * JMM : (q + 1) * JMM],
                                lhsT,
                                wstack[:, j0 : j0 + JMM],
                                start=True,
                                stop=True,
                            )
                        spool = spool_a if slot % 2 == 0 else spool_d
                        scr = spool.tile([128, JSUP], bf16, tag="scr")
                        if slot % 2 == 0:
                            nc.scalar.activation(
                                scr[:, off:],
                                ps[:, off:],
                                mybir.ActivationFunctionType.Relu,
                                bias=bias,
                                scale=1.0,
                                accum_out=acc_act[:, slot // 2 : slot // 2 + 1],
                            )
                        else:
                            nc.vector.scalar_tensor_tensor(
                                scr[:, off:],
                                ps[:, off:],
                                bias,
                                zeros[:, off:],
                                op0=mybir.AluOpType.add,
                                op1=mybir.AluOpType.max,
                                accum_out=acc_dve[:, slot // 2 : slot // 2 + 1],
                            )
                        slot += 1
                if n_slots % 2 == 1:
                    # odd slot count leaves the last dve column unwritten
                    nc.vector.memset(acc_dve[:, half - 1 : half], 0.0)
                nc.sync.dma_start(part_d[:, :half], acc_act[:])
                nc.sync.dma_start(part_d[:, half:], acc_dve[:])

    nc.compile()
    return nc


def _prep(preds, durations, events):
    """Host-side marshalling: sort by duration, build the one-hot/mask/bias
    operands, fp16 hi/lo split, and the O(B) scalar terms."""
    p = np.clip(np.asarray(preds, dtype=np.float32), 1e-12, 1.0 - 1e-12)
    dur = np.asarray(durations)
    ev = np.asarray(events, dtype=np.float32)
    Bn, Tn = p.shape

    d = np.clip(dur.astype(np.int64) - 1, 0, Tn - 1)
    t = p[np.arange(Bn), d]

    # O(B) host terms
    lik_sum = float(np.sum(-np.log(t.astype(np.float64)) * ev.astype(np.float64)))
    hist = np.bincount(d, minlength=Tn)
    gtc = np.zeros(Tn, np.int64)
    gtc[:-1] = hist[::-1].cumsum()[::-1][1:]  # gtc[c] = #{j : d_j > c}
    count = int((ev.astype(np.int64) * gtc[d]).sum())

    # sort rows by duration (stable); the j side keeps all rows
    order = np.argsort(d, kind="stable")
    d_s = d[order]
    p_s = p[order]

    cbins = np.arange(Tn)
    Wm = np.where(d_s[None, :] > cbins[:, None], p_s.T, np.float32(0.0)).astype(
        np.float32
    )
    Whi = Wm.astype(np.float16)
    Wlo = (Wm - Whi.astype(np.float32)).astype(np.float16)
    wstack_full = np.concatenate([Whi, Wlo], axis=0)  # [128, B]

    # i side: event rows only, sorted order, dealt round-robin to cores
    ev_s = ev[order]
    t_s = t[order]
    ev_pos = np.nonzero(ev_s == 1)[0]
    nev = len(ev_pos)

    ev_per_core = (nev + NCORES - 1) // NCORES
    n_itiles = max(1, (ev_per_core + ITILE - 1) // ITILE)
    blk = n_itiles * ITILE

    # per-core sorted event durations / thresholds, padded with d=Tn, t=0
    d_i = np.full((NCORES, blk), Tn, np.int64)
    t_i = np.zeros((NCORES, blk), np.float32)
    for c in range(NCORES):
        pos = ev_pos[c::NCORES]
        d_i[c, : len(pos)] = d_s[pos]
        t_i[c, : len(pos)] = t_s[pos]

    # first_gt[c] = first sorted j with d_j > c
    first_gt = np.searchsorted(d_s, np.arange(Tn), side="right")

    # per i-tile j extent (max over cores, rounded up to JMM=512)
    jlims = []
    for tt in range(n_itiles):
        need = JMM
        for c in range(NCORES):
            dmin = int(d_i[c, tt * ITILE : (tt + 1) * ITILE].min())
            if dmin < Tn:
                n = Bn - int(first_gt[dmin])
                need = max(need, ((n + JMM - 1) // JMM) * JMM)
        jlims.append(min(need, Bn))
    # wstack width must tile evenly into JSUP supers
    jmax = min(((max(jlims) + JSUP - 1) // JSUP) * JSUP, Bn)

    # device wstack holds the last jmax sorted rows
    wstack = np.ascontiguousarray(wstack_full[:, Bn - jmax :])

    in_maps = []
    for c in range(NCORES):
        E = (d_i[c][None, :] == cbins[:, None]).astype(np.float16)  # [T, blk]
        estack = np.ascontiguousarray(np.concatenate([E, E], axis=0))
        negt = np.ascontiguousarray(
            (-t_i[c]).reshape(n_itiles, ITILE).T
        )  # [128, n_itiles]
        in_maps.append({"wstack": wstack, "estack": estack, "negt": negt})
    return in_maps, n_itiles, jlims, lik_sum, count, Bn


def kernel(preds, durations, events):
    in_maps, n_itiles, jlims, lik_sum, count, Bn = _prep(preds, durations, events)

    key = (n_itiles, tuple(jlims))
    if key not in _cache:
        _cache[key] = _build_program(n_itiles, jlims)
    nc = _cache[key]

    res = run_bass_kernel_spmd(nc, in_maps, core_ids=list(range(NCORES)))
    rank_sum = 0.0
    for r in res.results:
        rank_sum += float(r["partials"].astype(np.float64).sum())

    rank = rank_sum / count if count > 0 else 0.0
    total = 0.5 * (lik_sum / Bn) + 0.5 * rank
    return np.array(total, dtype=np.float32)


# revision 17
# speedup vs baseline: 23.6029x; 1.4815x over previous
"""DeepHit loss kernel for Trainium2 (8 NeuronCores, Bass/Tile).

Math
----
reference:
    p   = clip(preds, 1e-12, 1-1e-12)            [B, T]
    d_i = clip(durations_i - 1, 0, T-1)
    t_i = p[i, d_i]
    lik = -log(t_i) * ev_i                       (weights are all 1.0)
    rank_sum = sum_{i,j} relu(p[j, d_i] - t_i) * [d_j > d_i] * [ev_i = 1]
    count    = #{(i,j) : d_j > d_i, ev_i = 1}
    out = 0.5 * mean(lik) + 0.5 * rank_sum / count

Device reformulation (the only O(B^2) term is rank_sum):
    durations take T=64 distinct values, so the gather p[j, d_i] is a
    one-hot matmul.  Let
        Wm[c, j] = p[j, c] * [d_j > c]            (mask folded into columns)
        E [c, i] = [d_i == c]                     (one-hot; i = event rows)
        bias_i   = -t_i
    then
        relu((E^T @ Wm)[i, j] + bias_i)
          = relu(p[j,d_i] * [d_j > d_i] - t_i)
          = relu(p[j,d_i] - t_i) * [d_j > d_i]
    because t_i > 0 makes the masked case (-t_i) vanish under relu.
    rank_sum is the global sum of that matrix over event rows i.

    On device: i on PSUM partitions (so bias_i is a per-partition ACT bias),
    j on the free dim.  The matmul runs in fp16 hi/lo split (exact to
    ~2^-24 relative since E is exactly representable): K packs [Whi; Wlo]
    against [E; E] so one K=128 matmul does hi+lo in a single pass.
    The relu+bias+sum runs fused on ScalarE (activation accum_out) and
    VectorE (scalar_tensor_tensor accum_out), alternating tiles.

Sharding + work reduction:
    * only event rows appear on the i side (~B/2 of them);
    * rows are sorted by duration (host permutation; rank_sum and count
      are permutation invariant) and event rows are dealt round-robin to
      the 8 cores so every core sees the same duration profile;
    * per i-tile, j is restricted to the suffix of sorted rows with
      d_j > min(d_i of the tile); the per-element Wm mask keeps the
      boundary exact.
    Each core returns [128, n_slots] partial sums; the host adds them and
    combines with the O(B) NLL/count terms (host-side marshalling only).
"""

import sys

sys.path.insert(0, "/opt/trn_rl_repo")

import numpy as np

import concourse.bacc as bacc
import concourse.mybir as mybir
import concourse.tile as tile
from concourse.bass_utils import run_bass_kernel_spmd

B = 8192
T = 64
NCORES = 8
ITILE = 128          # i rows per PSUM tile (partition dim)
JSUP = 1024          # j columns per consume instruction (2 PSUM banks)
JMM = 512            # j columns per matmul (1 PSUM bank)
PSUM_BUFS = 8 // (JSUP // 512)   # use all 8 banks; >=2 bufs per consumer

f16 = mybir.dt.float16
f32 = mybir.dt.float32
bf16 = mybir.dt.bfloat16

_cache = {}


def _build_program(n_itiles, jlims, repeat=1):
    """Build + compile the SPMD bass program.

    n_itiles: i-tiles per core (each 128 partitions).
    jlims[t]: j extent (multiple of JMM) needed by i-tile t; the tile's
              matmuls cover the LAST jlims[t] columns of wstack.  The
              structure is identical on every core; cores differ in data.
    repeat:   emit the whole body N times (timing variants only).
    """
    nc = bacc.Bacc(
        "TRN2", target_bir_lowering=False, debug=False, num_devices=NCORES
    )

    blk = n_itiles * ITILE
    # must match _prep's wstack width (rounded up to whole supers)
    jmax = ((max(jlims) + JSUP - 1) // JSUP) * JSUP
    # per tile: list of (super_base, offset_within_super); the consume
    # covers columns [super_base + offset, super_base + JSUP)
    tile_supers = []
    n_slots = 0
    for jl in jlims:
        start = jmax - jl  # multiple of JMM
        sbase0 = (start // JSUP) * JSUP
        supers = []
        for sbase in range(sbase0, jmax, JSUP):
            off = max(0, start - sbase)
            supers.append((sbase, off))
        tile_supers.append(supers)
        n_slots += len(supers)
    half = (n_slots + 1) // 2

    wstack_d = nc.dram_tensor("wstack", [128, jmax], f16, kind="ExternalInput")
    estack_d = nc.dram_tensor("estack", [128, blk], f16, kind="ExternalInput")
    negt_d = nc.dram_tensor("negt", [128, n_itiles], f32, kind="ExternalInput")
    part_d = nc.dram_tensor("partials", [128, 2 * half], f32, kind="ExternalOutput")

    with tile.TileContext(nc) as tc:
        with (
            tc.tile_pool(name="const", bufs=1) as zpool,
            tc.tile_pool(name="inp", bufs=min(2, max(1, repeat))) as cpool,
            tc.tile_pool(name="psum", bufs=PSUM_BUFS, space="PSUM") as ppool,
            tc.tile_pool(name="scr_a", bufs=2) as spool_a,
            tc.tile_pool(name="scr_d", bufs=2) as spool_d,
        ):
            zeros = zpool.tile([128, JSUP], bf16)
            nc.vector.memset(zeros[:], 0.0)
            # dummy activation with no data deps: pulls the ~2.7us Relu
            # table load to kernel start, hidden under the input DMA
            warm = zpool.tile([128, 1], f32)
            nc.scalar.activation(
                warm[:], zeros[:, :1], mybir.ActivationFunctionType.Relu
            )

            for _rep in range(repeat):
                # one SEPARATE tile per wstack chunk so a matmul depends
                # only on its own chunk's DMA (a single tile would make
                # every matmul wait for the whole 2 MB load).  High j
                # (large durations) first: every i-tile starts there.
                # small inputs first (the first matmul needs estack+negt),
                # then chunks high-j-first: every i-tile starts there.
                estack = cpool.tile([128, blk], f16, tag="estack")
                nc.sync.dma_start(estack[:], estack_d[:])
                negt = cpool.tile([128, n_itiles], f32, tag="negt")
                nc.sync.dma_start(negt[:], negt_d[:])
                wchunks = {}
                for j0 in reversed(range(0, jmax, JSUP)):
                    wc = cpool.tile([128, JSUP], f16, tag=f"wst{j0}")
                    nc.sync.dma_start(wc[:], wstack_d[:, j0 : j0 + JSUP])
                    wchunks[j0] = wc
                # separate accumulator tiles per engine so ACT/DVE never
                # share a written tile (keeps the two streams independent)
                acc_act = cpool.tile([128, half], f32, tag="acc_act")
                acc_dve = cpool.tile([128, half], f32, tag="acc_dve")

                slot = 0
                for it in range(n_itiles):
                    lhsT = estack[:, it * ITILE : (it + 1) * ITILE]
                    bias = negt[:, it : it + 1]
                    # high j first: those wstack chunks are DMA'd first, and
                    # every i-tile needs the high end
                    for sbase, off in reversed(tile_supers[it]):
                        ps = ppool.tile([128, JSUP], f32, tag="ps")
                        for q in range(off // JMM, JSUP // JMM):
                            nc.tensor.matmul(
                                ps[:, q * JMM : (q + 1) * JMM],
                                lhsT,
                                wchunks[sbase][:, q * JMM : (q + 1) * JMM],
                                start=True,
                                stop=True,
                            )
                        spool = spool_a if slot % 2 == 0 else spool_d
                        scr = spool.tile([128, JSUP], bf16, tag="scr")
                        if slot % 2 == 0:
                            nc.scalar.activation(
                                scr[:, off:],
                                ps[:, off:],
                                mybir.ActivationFunctionType.Relu,
                                bias=bias,
                                scale=1.0,
                                accum_out=acc_act[:, slot // 2 : slot // 2 + 1],
                            )
                        else:
                            nc.vector.scalar_tensor_tensor(
                                scr[:, off:],
                                ps[:, off:],
                                bias,
                                zeros[:, off:],
                                op0=mybir.AluOpType.add,
                                op1=mybir.AluOpType.max,
                                accum_out=acc_dve[:, slot // 2 : slot // 2 + 1],
                            )
                        slot += 1
                if n_slots % 2 == 1:
                    # odd slot count leaves the last dve column unwritten
                    nc.vector.memset(acc_dve[:, half - 1 : half], 0.0)
                nc.sync.dma_start(part_d[:, :half], acc_act[:])
                nc.sync.dma_start(part_d[:, half:], acc_dve[:])

    nc.compile()
    return nc


def _prep(preds, durations, events):
    """Host-side marshalling: sort by duration, build the one-hot/mask/bias
    operands, fp16 hi/lo split, and the O(B) scalar terms."""
    p = np.clip(np.asarray(preds, dtype=np.float32), 1e-12, 1.0 - 1e-12)
    dur = np.asarray(durations)
    ev = np.asarray(events, dtype=np.float32)
    Bn, Tn = p.shape

    d = np.clip(dur.astype(np.int64) - 1, 0, Tn - 1)
    t = p[np.arange(Bn), d]

    # O(B) host terms
    lik_sum = float(np.sum(-np.log(t.astype(np.float64)) * ev.astype(np.float64)))
    hist = np.bincount(d, minlength=Tn)
    gtc = np.zeros(Tn, np.int64)
    gtc[:-1] = hist[::-1].cumsum()[::-1][1:]  # gtc[c] = #{j : d_j > c}
    count = int((ev.astype(np.int64) * gtc[d]).sum())

    # sort rows by duration (stable); the j side keeps all rows
    order = np.argsort(d, kind="stable")
    d_s = d[order]
    p_s = p[order]

    cbins = np.arange(Tn)
    Wm = np.where(d_s[None, :] > cbins[:, None], p_s.T, np.float32(0.0)).astype(
        np.float32
    )
    Whi = Wm.astype(np.float16)
    Wlo = (Wm - Whi.astype(np.float32)).astype(np.float16)
    wstack_full = np.concatenate([Whi, Wlo], axis=0)  # [128, B]

    # i side: event rows only, sorted order, dealt round-robin to cores
    ev_s = ev[order]
    t_s = t[order]
    ev_pos = np.nonzero(ev_s == 1)[0]
    nev = len(ev_pos)

    ev_per_core = (nev + NCORES - 1) // NCORES
    n_itiles = max(1, (ev_per_core + ITILE - 1) // ITILE)
    blk = n_itiles * ITILE

    # per-core sorted event durations / thresholds, padded with d=Tn, t=0
    d_i = np.full((NCORES, blk), Tn, np.int64)
    t_i = np.zeros((NCORES, blk), np.float32)
    for c in range(NCORES):
        pos = ev_pos[c::NCORES]
        d_i[c, : len(pos)] = d_s[pos]
        t_i[c, : len(pos)] = t_s[pos]

    # first_gt[c] = first sorted j with d_j > c
    first_gt = np.searchsorted(d_s, np.arange(Tn), side="right")

    # per i-tile j extent (max over cores, rounded up to JMM=512)
    jlims = []
    for tt in range(n_itiles):
        need = JMM
        for c in range(NCORES):
            dmin = int(d_i[c, tt * ITILE : (tt + 1) * ITILE].min())
            if dmin < Tn:
                n = Bn - int(first_gt[dmin])
                need = max(need, ((n + JMM - 1) // JMM) * JMM)
        jlims.append(min(need, Bn))
    # wstack width must tile evenly into JSUP supers
    jmax = min(((max(jlims) + JSUP - 1) // JSUP) * JSUP, Bn)

    # device wstack holds the last jmax sorted rows
    wstack = np.ascontiguousarray(wstack_full[:, Bn - jmax :])

    in_maps = []
    for c in range(NCORES):
        E = (d_i[c][None, :] == cbins[:, None]).astype(np.float16)  # [T, blk]
        estack = np.ascontiguousarray(np.concatenate([E, E], axis=0))
        negt = np.ascontiguousarray(
            (-t_i[c]).reshape(n_itiles, ITILE).T
        )  # [128, n_itiles]
        in_maps.append({"wstack": wstack, "estack": estack, "negt": negt})
    return in_maps, n_itiles, jlims, lik_sum, count, Bn


def kernel(preds, durations, events):
    in_maps, n_itiles, jlims, lik_sum, count, Bn = _prep(preds, durations, events)

    key = (n_itiles, tuple(jlims))
    if key not in _cache:
        _cache[key] = _build_program(n_itiles, jlims)
    nc = _cache[key]

    res = run_bass_kernel_spmd(nc, in_maps, core_ids=list(range(NCORES)))
    rank_sum = 0.0
    for r in res.results:
        rank_sum += float(r["partials"].astype(np.float64).sum())

    rank = rank_sum / count if count > 0 else 0.0
    total = 0.5 * (lik_sum / Bn) + 0.5 * rank
    return np.array(total, dtype=np.float32)
